# revision 19
# baseline (speedup 1.0000x reference)
"""Trainium2 Bass kernel for nn_Extract_HyperSpherePrototypes.

Computation (see reference):
  1. L2-normalize each pixel's feature vector over the channel dim F=256.
  2. Segment-sum normalized features by label into [C+1=20, F] prototypes.
  3. Drop void class, transpose to [F, 19], L2-normalize each column.

Sharding: data-parallel over batch (16 items / 8 cores = 2 per core).
Each core computes a local [20, 256] partial, AllReduce(sum) across the
8 cores, then every core normalizes columns and writes the full output.

Per-core layout: features[b] is processed as two w-half tiles
[h=128; f=256, w=64] (partition dim = h). The per-pixel inverse norm is
folded into the one-hot matrix M[h, w, c] = (label==c) * rsqrt(sum_f x^2),
so the segment-sum becomes, per w column, one matmul contracting h with
a full-width (N=256) moving operand:
  protos[c, :] += M[:, w, :].T @ X[:, :, w]
N=256 lets the matmuls run as float32r (1 cycle/row vs 4 for fp32).
Set KERNEL_MM_DTYPE=f32 for full-precision fp32 matmuls instead.
"""

import os

import numpy as np

import concourse.bass as bass
import concourse.bacc as bacc
from concourse import mybir
from concourse.bass_utils import run_bass_kernel_spmd
from concourse.tile import TileContext

F32 = mybir.dt.float32
F32R = mybir.dt.float32r
AX = mybir.AxisListType
OP = mybir.AluOpType
ACT_FN = mybir.ActivationFunctionType

NCORES = 8
B_TOT = 16
BPC = B_TOT // NCORES  # batches per core
F = 256
H = 128
W = 128
C = 20  # 19 known + void
FC = 64  # f-chunk per tile
NFC = F // FC
WH = 64  # w-half for square scratch
QW = 4  # w-columns packed per matmul (lhsT = [h, QW*CP])
CP = 32  # class block padded to PSUM partition alignment
NQ = W // QW

EPS2 = 1e-24  # matches max(norm, 1e-12) in the reference

_NO_CC = bool(int(os.environ.get("KERNEL_NO_CC", "0")))
_MM_F32 = os.environ.get("KERNEL_MM_DTYPE", "f32r") == "f32"


def build_nc():
    mm_dt = F32 if _MM_F32 else F32R
    nc = bacc.Bacc("TRN2", target_bir_lowering=False)

    feats = nc.declare_dram_parameter("feats", [BPC, F, H, W], mm_dt, isOutput=False)
    labs = nc.declare_dram_parameter("labs", [BPC, H, W], F32, isOutput=False)
    out_d = nc.declare_dram_parameter("out", [F, C - 1], F32, isOutput=True)

    cc_in = nc.dram_tensor("cc_in", [C, F], F32)
    cc_out = nc.dram_tensor("cc_out", [C, F], F32, addr_space="Shared")

    with TileContext(nc) as tc:
        with (
            tc.tile_pool(name="consts", bufs=1) as consts,
            tc.tile_pool(name="xp", bufs=4) as xp,
            tc.tile_pool(name="sqp", bufs=1) as sqp,
            tc.tile_pool(name="mp", bufs=2) as mp,
            tc.tile_pool(name="normp", bufs=2) as normp,
            tc.tile_pool(name="finp", bufs=1) as finp,
            tc.tile_pool(name="psum", bufs=1, space="PSUM") as psum,
        ):
            iota_i = consts.tile([H, CP], mybir.dt.int32)
            nc.gpsimd.iota(iota_i, pattern=[[1, CP]], base=0, channel_multiplier=0)
            iota_sb = consts.tile([H, CP], F32)
            nc.vector.tensor_copy(iota_sb, iota_i)
            eps_sb = consts.tile([H, 1], F32)
            nc.vector.memset(eps_sb, EPS2)

            feats_ap = feats.ap()
            labs_ap = labs.ap()

            psq = []
            for fc in range(NFC):
                psq_t = psum.tile([QW * CP, FC * QW], F32, tag=f"ps{fc}")
                psq.append(psq_t)

            for b in range(BPC):
                lab_sb = normp.tile([H, W], F32)
                nc.sync.dma_start(out=lab_sb, in_=labs_ap[b])

                hfw = feats_ap[b].rearrange("f h w -> h f w")
                ssq4 = normp.tile([H, W, NFC], F32)
                xts = []
                for fc in range(NFC):
                    xt = xp.tile([H, FC, W], mm_dt)
                    nc.sync.dma_start(
                        out=xt, in_=hfw[:, fc * FC : (fc + 1) * FC, :]
                    )
                    xts.append(xt)
                    # sumsq over f per (h, w): square on ACT, reduce on DVE
                    for wh in range(W // WH):
                        sq = sqp.tile([H, FC, WH], F32)
                        src = xt[:, :, wh * WH : (wh + 1) * WH]
                        if not _MM_F32:
                            src = src.bitcast(F32)
                        nc.scalar.activation(out=sq, in_=src, func=ACT_FN.Square)
                        nc.vector.tensor_reduce(
                            out=ssq4[:, wh * WH : (wh + 1) * WH, fc],
                            in_=sq.rearrange("h f w -> h w f"),
                            axis=AX.X,
                            op=OP.add,
                        )
                ssq = normp.tile([H, W], F32)
                nc.vector.tensor_reduce(out=ssq, in_=ssq4, axis=AX.X, op=OP.add)
                nc.scalar.activation(out=ssq, in_=ssq, func=ACT_FN.Sqrt, bias=eps_sb[:])
                inv = normp.tile([H, W], F32)
                nc.vector.reciprocal(out=inv, in_=ssq)

                # M[h, w, c] = (iota_c == lab) * inv   (rounded to mm dtype);
                # c padded to CP=32 so PSUM class blocks are partition-aligned
                m_sb = mp.tile([H, W, CP], mm_dt)
                m_f32 = m_sb[:]
                nc.vector.tensor_tensor(
                    out=m_f32,
                    in0=bass.AP(
                        tensor=iota_sb[:].tensor,
                        offset=iota_sb[:].offset,
                        ap=[iota_sb[:].ap[0], [0, W], [1, CP]],
                    ),
                    in1=lab_sb[:].to_broadcast([H, W, CP]),
                    op=OP.is_equal,
                )
                nc.vector.tensor_tensor(
                    out=m_sb,
                    in0=m_f32,
                    in1=inv[:].to_broadcast([H, W, CP]),
                    op=OP.mult,
                )

                # segment-sum, QW w-columns per matmul:
                #   psq[fc][wl*C + c, f*QW + wl'] += sum_h M[h, q*QW+wl, c] X[h, f, q*QW+wl']
                # diagonal wl == wl' blocks are the real contributions.
                for fc in range(NFC):
                    for q in range(NQ):
                        nc.tensor.matmul(
                            out=psq[fc],
                            lhsT=m_sb[:, q * QW : (q + 1) * QW, :].rearrange(
                                "h w c -> h (w c)"
                            ),
                            rhs=xts[fc][:, :, q * QW : (q + 1) * QW],
                            start=(b == 0 and q == 0),
                            stop=(b == BPC - 1 and q == NQ - 1),
                        )

            # combine diagonal blocks: protos[c, fc*FC + f] = sum_wl psq[fc][wl*C+c, f*QW+wl]
            protos_sb = finp.tile([C, F], F32)
            for fc in range(NFC):
                pv = psq[fc][:].rearrange("m (f w) -> m f w", w=QW)
                dst = protos_sb[:, fc * FC : (fc + 1) * FC]
                nc.scalar.copy(out=dst, in_=pv[0:C, :, 0])
                for wl in range(1, QW):
                    nc.vector.tensor_add(
                        dst, dst, pv[wl * CP : wl * CP + C, :, wl]
                    )
            if not _NO_CC:
                nc.sync.dma_start(out=cc_in.ap(), in_=protos_sb)
                nc.gpsimd.collective_compute(
                    "AllReduce",
                    OP.add,
                    ins=[cc_in.ap().opt()],
                    outs=[cc_out.ap().opt()],
                    replica_groups=[list(range(NCORES))],
                )
                red_sb = finp.tile([C, F], F32)
                nc.sync.dma_start(out=red_sb, in_=cc_out.ap())
            else:
                red_sb = protos_sb

            # column norms (per class over F): pn2[c] = sum_f red[c,f]^2
            scr = finp.tile([C, F], F32)
            pn = finp.tile([C, 1], F32)
            nc.vector.tensor_mul(scr, red_sb, red_sb)
            nc.vector.tensor_reduce(out=pn, in_=scr, axis=AX.X, op=OP.add)
            nc.scalar.activation(out=pn, in_=pn, func=ACT_FN.Sqrt, bias=eps_sb[:C])
            pninv = finp.tile([C, 1], F32)
            nc.vector.reciprocal(out=pninv, in_=pn)
            nc.vector.tensor_scalar_mul(out=red_sb, in0=red_sb, scalar1=pninv)

            # transposed write: out[f, c] = red_sb[c, f]
            o_ap = out_d.ap()
            nc.sync.dma_start(
                out=bass.AP(
                    tensor=o_ap.tensor,
                    offset=o_ap.offset,
                    ap=[[1, C - 1], [C - 1, F]],
                ),
                in_=red_sb[0 : C - 1, :],
            )

    nc.compile()
    return nc


_NC_CACHE = None


def _get_nc():
    global _NC_CACHE
    if _NC_CACHE is None:
        _NC_CACHE = build_nc()
    return _NC_CACHE


def kernel(features: np.ndarray, labels: np.ndarray) -> np.ndarray:
    features = np.ascontiguousarray(np.asarray(features, dtype=np.float32))
    labs_f32 = np.asarray(labels, dtype=np.float32)  # values 0..19, exact in f32

    nc = _get_nc()
    in_maps = []
    for core in range(NCORES):
        in_maps.append(
            {
                "feats": features[core * BPC : (core + 1) * BPC],
                "labs": np.ascontiguousarray(labs_f32[core * BPC : (core + 1) * BPC]),
            }
        )
    res = run_bass_kernel_spmd(nc, in_maps, core_ids=list(range(NCORES)))
    return np.asarray(res.results[0]["out"], dtype=np.float32)


# revision 20
# speedup vs baseline: 42292.1903x; 42292.1903x over previous
"""Trainium2 Bass kernel for nn_Extract_HyperSpherePrototypes.

Computation (see reference):
  1. L2-normalize each pixel's feature vector over the channel dim F=256.
  2. Segment-sum normalized features by label into [C+1=20, F] prototypes.
  3. Drop void class, transpose to [F, 19], L2-normalize each column.

Sharding: data-parallel over batch (16 items / 8 cores = 2 per core).
Each core computes a local [20, 256] partial, AllReduce(sum) across the
8 cores, then every core normalizes columns and writes the full output.

Per-core layout: features[b] is loaded as four f-chunk tiles
[h=128; f=64, w=128] (partition = h, w innermost) so every DMA burst is a
contiguous 512B run (full line rate). The per-pixel inverse norm is folded
into a one-hot matrix M[h, w, c] = (label==c) * rsqrt(sum_f x^2), so the
segment-sum contracts h on the tensor engine. To keep the moving operand
at N=256 (float32r matmuls run 1 cycle/row there vs 4 for fp32), each
matmul packs QW=4 w-columns: lhsT stacks 4 w's masks in 32-partition
blocks (padded for PSUM alignment), rhs spans [64 f x 4 w]; the cross
(wl != wl') blocks land in distinct PSUM columns and are dropped by the
final diagonal-block combine. Set KERNEL_MM_DTYPE=f32 for full-precision
fp32 matmuls (~4x slower PE, ~3.4e-6 rel err vs ~1.4e-4 for f32r).
"""

import os

import numpy as np

import concourse.bass as bass
import concourse.bacc as bacc
from concourse import mybir
from concourse.bass_utils import run_bass_kernel_spmd
from concourse.tile import TileContext

F32 = mybir.dt.float32
F32R = mybir.dt.float32r
AX = mybir.AxisListType
OP = mybir.AluOpType
ACT_FN = mybir.ActivationFunctionType

NCORES = 8
B_TOT = 16
BPC = B_TOT // NCORES  # batches per core
F = 256
H = 128
W = 128
C = 20  # 19 known + void
FC = 64  # f-chunk per tile
NFC = F // FC
WH = 64  # w-half for square scratch
QW = 4  # w-columns packed per matmul (lhsT = [h, QW*CP])
CP = 32  # class block padded to PSUM partition alignment
NQ = W // QW

EPS2 = 1e-24  # matches max(norm, 1e-12) in the reference

_NO_CC = bool(int(os.environ.get("KERNEL_NO_CC", "0")))
_MM_F32 = os.environ.get("KERNEL_MM_DTYPE", "f32r") == "f32"


def build_nc():
    mm_dt = F32 if _MM_F32 else F32R
    nc = bacc.Bacc("TRN2", target_bir_lowering=False)

    feats = nc.declare_dram_parameter("feats", [BPC, F, H, W], mm_dt, isOutput=False)
    labs = nc.declare_dram_parameter("labs", [BPC, H, W], F32, isOutput=False)
    out_d = nc.declare_dram_parameter("out", [F, C - 1], F32, isOutput=True)

    cc_in = nc.dram_tensor("cc_in", [C, F], F32)
    cc_out = nc.dram_tensor("cc_out", [C, F], F32, addr_space="Shared")

    with TileContext(nc) as tc:
        with (
            tc.tile_pool(name="consts", bufs=1) as consts,
            tc.tile_pool(name="xp", bufs=4) as xp,
            tc.tile_pool(name="sqp", bufs=1) as sqp,
            tc.tile_pool(name="mp", bufs=2) as mp,
            tc.tile_pool(name="normp", bufs=2) as normp,
            tc.tile_pool(name="finp", bufs=1) as finp,
            tc.tile_pool(name="psum", bufs=1, space="PSUM") as psum,
        ):
            iota_i = consts.tile([H, CP], mybir.dt.int32)
            nc.gpsimd.iota(iota_i, pattern=[[1, CP]], base=0, channel_multiplier=0)
            iota_sb = consts.tile([H, CP], F32)
            nc.vector.tensor_copy(iota_sb, iota_i)
            eps_sb = consts.tile([H, 1], F32)
            nc.vector.memset(eps_sb, EPS2)

            feats_ap = feats.ap()
            labs_ap = labs.ap()

            psq = []
            for fc in range(NFC):
                psq_t = psum.tile([QW * CP, FC * QW], F32, tag=f"ps{fc}")
                psq.append(psq_t)

            for b in range(BPC):
                lab_sb = normp.tile([H, W], F32)
                nc.sync.dma_start(out=lab_sb, in_=labs_ap[b])

                hfw = feats_ap[b].rearrange("f h w -> h f w")
                ssq4 = normp.tile([H, W, NFC], F32)
                xts = []
                for fc in range(NFC):
                    xt = xp.tile([H, FC, W], mm_dt)
                    nc.sync.dma_start(
                        out=xt, in_=hfw[:, fc * FC : (fc + 1) * FC, :]
                    )
                    xts.append(xt)
                    # sumsq over f per (h, w): square on ACT, reduce on DVE
                    for wh in range(W // WH):
                        sq = sqp.tile([H, FC, WH], F32)
                        src = xt[:, :, wh * WH : (wh + 1) * WH]
                        if not _MM_F32:
                            src = src.bitcast(F32)
                        nc.scalar.activation(out=sq, in_=src, func=ACT_FN.Square)
                        nc.vector.tensor_reduce(
                            out=ssq4[:, wh * WH : (wh + 1) * WH, fc],
                            in_=sq.rearrange("h f w -> h w f"),
                            axis=AX.X,
                            op=OP.add,
                        )
                ssq = normp.tile([H, W], F32)
                nc.vector.tensor_reduce(out=ssq, in_=ssq4, axis=AX.X, op=OP.add)
                nc.scalar.activation(out=ssq, in_=ssq, func=ACT_FN.Sqrt, bias=eps_sb[:])
                inv = normp.tile([H, W], F32)
                nc.vector.reciprocal(out=inv, in_=ssq)

                # M[h, w, c] = (iota_c == lab) * inv   (rounded to mm dtype);
                # c padded to CP=32 so PSUM class blocks are partition-aligned
                m_sb = mp.tile([H, W, CP], mm_dt)
                m_f32 = m_sb[:]
                nc.vector.tensor_tensor(
                    out=m_f32,
                    in0=bass.AP(
                        tensor=iota_sb[:].tensor,
                        offset=iota_sb[:].offset,
                        ap=[iota_sb[:].ap[0], [0, W], [1, CP]],
                    ),
                    in1=lab_sb[:].to_broadcast([H, W, CP]),
                    op=OP.is_equal,
                )
                nc.vector.tensor_tensor(
                    out=m_sb,
                    in0=m_f32,
                    in1=inv[:].to_broadcast([H, W, CP]),
                    op=OP.mult,
                )

                # segment-sum, QW w-columns per matmul:
                #   psq[fc][wl*C + c, f*QW + wl'] += sum_h M[h, q*QW+wl, c] X[h, f, q*QW+wl']
                # diagonal wl == wl' blocks are the real contributions.
                for fc in range(NFC):
                    for q in range(NQ):
                        nc.tensor.matmul(
                            out=psq[fc],
                            lhsT=m_sb[:, q * QW : (q + 1) * QW, :].rearrange(
                                "h w c -> h (w c)"
                            ),
                            rhs=xts[fc][:, :, q * QW : (q + 1) * QW],
                            start=(b == 0 and q == 0),
                            stop=(b == BPC - 1 and q == NQ - 1),
                        )

            # combine diagonal blocks: protos[c, fc*FC + f] = sum_wl psq[fc][wl*C+c, f*QW+wl]
            protos_sb = finp.tile([C, F], F32)
            for fc in range(NFC):
                pv = psq[fc][:].rearrange("m (f w) -> m f w", w=QW)
                dst = protos_sb[:, fc * FC : (fc + 1) * FC]
                nc.scalar.copy(out=dst, in_=pv[0:C, :, 0])
                for wl in range(1, QW):
                    nc.vector.tensor_add(
                        dst, dst, pv[wl * CP : wl * CP + C, :, wl]
                    )
            if not _NO_CC:
                nc.sync.dma_start(out=cc_in.ap(), in_=protos_sb)
                nc.gpsimd.collective_compute(
                    "AllReduce",
                    OP.add,
                    ins=[cc_in.ap().opt()],
                    outs=[cc_out.ap().opt()],
                    replica_groups=[list(range(NCORES))],
                )
                red_sb = finp.tile([C, F], F32)
                nc.sync.dma_start(out=red_sb, in_=cc_out.ap())
            else:
                red_sb = protos_sb

            # column norms (per class over F): pn2[c] = sum_f red[c,f]^2
            scr = finp.tile([C, F], F32)
            pn = finp.tile([C, 1], F32)
            nc.vector.tensor_mul(scr, red_sb, red_sb)
            nc.vector.tensor_reduce(out=pn, in_=scr, axis=AX.X, op=OP.add)
            nc.scalar.activation(out=pn, in_=pn, func=ACT_FN.Sqrt, bias=eps_sb[:C])
            pninv = finp.tile([C, 1], F32)
            nc.vector.reciprocal(out=pninv, in_=pn)
            nc.vector.tensor_scalar_mul(out=red_sb, in0=red_sb, scalar1=pninv)

            # transposed write: out[f, c] = red_sb[c, f]
            o_ap = out_d.ap()
            nc.sync.dma_start(
                out=bass.AP(
                    tensor=o_ap.tensor,
                    offset=o_ap.offset,
                    ap=[[1, C - 1], [C - 1, F]],
                ),
                in_=red_sb[0 : C - 1, :],
            )

    nc.compile()
    return nc


_NC_CACHE = None


def _get_nc():
    global _NC_CACHE
    if _NC_CACHE is None:
        _NC_CACHE = build_nc()
    return _NC_CACHE


def kernel(features: np.ndarray, labels: np.ndarray) -> np.ndarray:
    features = np.ascontiguousarray(np.asarray(features, dtype=np.float32))
    labs_f32 = np.asarray(labels, dtype=np.float32)  # values 0..19, exact in f32

    nc = _get_nc()
    in_maps = []
    for core in range(NCORES):
        in_maps.append(
            {
                "feats": features[core * BPC : (core + 1) * BPC],
                "labs": np.ascontiguousarray(labs_f32[core * BPC : (core + 1) * BPC]),
            }
        )
    res = run_bass_kernel_spmd(nc, in_maps, core_ids=list(range(NCORES)))
    return np.asarray(res.results[0]["out"], dtype=np.float32)
